# revision 19
# baseline (speedup 1.0000x reference)
"""Trainium2 Bass kernel: fused MHA block (LN -> QKV -> q/k per-token LN ->
RoPE -> SDPA -> out-proj), SPMD over 8 NeuronCores.

Sharding: core c handles batch b = c//4, query-token quarter s = c%4. The host
rotates tokens (np.roll) per core so each core's query tokens are always
tokens [0, 512) of its input; attention keys/values cover all 2048 tokens
(softmax is permutation-invariant over keys). Host concatenates 8 [512, 1024]
output slices.

v2 pipeline (vs v1): the first layernorm is folded into the QKV weights on
the host (W' = W * ln_w, c = W @ ln_b, s = rowsum(W')), so projections run
directly from raw xT with no hT materialization:
  qkv[t, e] = r_t * (x_t @ W'^T)[e] - (mu_t * r_t) * s_e + c_e
The per-token x-stats (mu, rstd) are computed concurrently via ones-matmuls
and bounced through DRAM into per-token COLUMNS [P, 1] for the token-major
affine fixup (ACT Identity with per-partition scale + Pool/DVE correction).
Weights prefetch on separate queues; XBAR transposes issue on the SP queue;
softmax denominators are broadcast per head-pair (one DRAM bounce per et).
"""

import numpy as np
import ml_dtypes

import concourse.bass as bass
import concourse.mybir as mybir
import concourse.tile as tile
from concourse import bacc
from concourse.bass_utils import run_bass_kernel_spmd

B, L, D, H, DH = 2, 2048, 1024, 16, 64
EPS = 1e-5
ROPE_BASE = 10000.0
NCORES = 8
LQ = L // 4
P = 128
ND = D // P      # 8 feature tiles of 128
NT = L // P      # 16 token tiles
NTQ = LQ // P    # 4 query token tiles
FD = 512         # psum bank free size (f32)
NSL = L // FD    # 4 slabs of keys/tokens
BF = mybir.dt.bfloat16
F32 = mybir.dt.float32
AF = mybir.ActivationFunctionType
OP = mybir.AluOpType


def _bc_part(ap, parts):
    """Partition-broadcast (step 0) of a [1, ...] DRAM AP to `parts` rows."""
    return bass.AP(tensor=ap.tensor, offset=ap.offset,
                   ap=[[0, parts]] + list(ap.ap[1:]))


def _bc_heads(ap2, n, at=1):
    """Insert a step-0 dim of size n at free position `at` of a 2D sbuf AP."""
    dims = list(ap2.ap)
    return bass.AP(tensor=ap2.tensor, offset=ap2.offset,
                   ap=dims[:at] + [[0, n]] + dims[at:])


def _emit(nc):
    xT = nc.dram_tensor("xT", [D, L], BF, kind="ExternalInput")
    wqkvT = nc.dram_tensor("wqkvT", [D, 3 * D], BF, kind="ExternalInput")
    woutT = nc.dram_tensor("woutT", [D, D], BF, kind="ExternalInput")
    sneg = nc.dram_tensor("sneg", [3 * D], BF, kind="ExternalInput")
    crow = nc.dram_tensor("crow", [3 * D], BF, kind="ExternalInput")
    q_ln_w = nc.dram_tensor("q_ln_w", [D], BF, kind="ExternalInput")
    k_ln_w = nc.dram_tensor("k_ln_w", [D], BF, kind="ExternalInput")
    cos_t = nc.dram_tensor("cos_t", [L, DH], BF, kind="ExternalInput")
    sin_t = nc.dram_tensor("sin_t", [L, DH], BF, kind="ExternalInput")
    out = nc.dram_tensor("out", [LQ, D], F32, kind="ExternalOutput")

    with tile.TileContext(nc) as tc:
        _body(nc, tc, xT, wqkvT, woutT, sneg, crow, q_ln_w, k_ln_w,
              cos_t, sin_t, out)
    return nc


def _rstd_refine(nc, pool, r, vareps, shape, name, zb_t, os_t):
    """One Newton step for r ~= rsqrt(varep): r' = r*(1.5 - 0.5*varep*r^2).
    Guards against ACT sqrt LUT error on hardware. In-place on r."""
    t = pool.tile(list(shape), F32, name=f"{name}_nt", tag=f"{name}_nt", bufs=2)
    nc.scalar.activation(t[:], r[:], AF.Square, bias=zb_t[0:shape[0], :],
                         scale=os_t[0:shape[0], :])
    nc.vector.tensor_mul(t[:], t[:], vareps[:])
    nc.vector.tensor_scalar(t[:], t[:], -0.5, 1.5, op0=OP.mult, op1=OP.add)
    nc.vector.tensor_mul(r[:], r[:], t[:])


def _body(nc, tc, xT, wqkvT, woutT, sneg, crow, q_ln_w, k_ln_w,
          cos_t, sin_t, out):
    import contextlib
    ap_xT = xT.ap().rearrange("(nd p) t -> p nd t", p=P)
    ap_wqkvT = wqkvT.ap().rearrange("(nd p) e -> p nd e", p=P)
    ap_woutT = woutT.ap().rearrange("(nd p) e -> p nd e", p=P)
    ap_cos = cos_t.ap().rearrange("(tt p) j -> p tt j", p=P)
    ap_sin = sin_t.ap().rearrange("(tt p) j -> p tt j", p=P)

    ctx = contextlib.ExitStack()
    with ctx:
        const = ctx.enter_context(tc.tile_pool(name="const", bufs=1))
        live = ctx.enter_context(tc.tile_pool(name="live", bufs=1))
        stat = ctx.enter_context(tc.tile_pool(name="stat", bufs=1))
        dram = ctx.enter_context(tc.tile_pool(name="dram", bufs=2, space="DRAM"))

        # ---------- constants ----------
        qw_sb = const.tile([P, D], BF)      # q_ln_w broadcast to all partitions
        nc.gpsimd.dma_start(qw_sb[:], _bc_part(q_ln_w.ap()[None, :], P))
        kw_sb = const.tile([P, D], BF)
        nc.gpsimd.dma_start(kw_sb[:], _bc_part(k_ln_w.ap()[None, :], P))
        cos_sb = const.tile([P, NT, DH], BF)
        nc.gpsimd.dma_start(cos_sb[:], ap_cos)
        sin_sb = const.tile([P, NT, DH], BF)
        nc.gpsimd.dma_start(sin_sb[:], ap_sin)
        ones_sb = const.tile([P, 1], BF)
        nc.vector.memset(ones_sb[:], 1.0)
        # explicit zero-bias / one-scale consts: keeps bass from lazily
        # allocating implicit const tensors inside scoped-pool SBUF space
        zb_sb = const.tile([P, 1], F32)
        nc.vector.memset(zb_sb[:], 0.0)
        os_sb = const.tile([P, 1], F32)
        nc.vector.memset(os_sb[:], 1.0)
        zb, os_ = zb_sb[:], os_sb[:]
        zb1, os1 = zb_sb[0:1, :], os_sb[0:1, :]
        sneg_sb = const.tile([P, 3 * D], BF)   # -colsum(W') bc to partitions
        nc.gpsimd.dma_start(sneg_sb[:], _bc_part(sneg.ap()[None, :], P))
        c_sb = const.tile([P, 3 * D], BF)      # ln_b @ W^T bc to partitions
        nc.gpsimd.dma_start(c_sb[:], _bc_part(crow.ap()[None, :], P))

        # ---------- long-lived tensors ----------
        xT_sb = live.tile([P, ND, L], BF)
        for d in range(ND):
            nc.sync.dma_start(xT_sb[:, d, 0:FD], ap_xT[:, d, 0:FD])
        for sl in range(1, NSL):
            nc.sync.dma_start(xT_sb[:, :, sl * FD:(sl + 1) * FD],
                              ap_xT[:, :, sl * FD:(sl + 1) * FD])
        # weight slabs share a 3-deep slot set: wq, wk, wv live together in
        # phase A/B; wo recycles wq's slot after the q tiles finish.
        wpool = ctx.enter_context(tc.tile_pool(name="wpool", bufs=1))
        wq_sb = wpool.tile([P, ND, D], BF, name="wq", tag="wslab", bufs=2)
        nc.gpsimd.dma_start(wq_sb[:], ap_wqkvT[:, :, 0:D])
        wk_sb = wpool.tile([P, ND, D], BF, name="wk", tag="wslab", bufs=2)
        nc.sync.dma_start(wk_sb[:], ap_wqkvT[:, :, D:2 * D])

        v_sb = live.tile([P, NT, H, DH + 1], BF)
        nc.vector.memset(v_sb[:, :, :, DH:DH + 1], 1.0)
        krT = live.tile([P, ND, L], BF)
        qrT = live.tile([P, ND, LQ], BF)
        ctxT = live.tile([P, ND, LQ], BF)

        qr_d = dram.tile([LQ, D], BF, bufs=1)
        kr_d = dram.tile([L, D], BF, bufs=1)
        rows_d = dram.tile([2, L], F32, bufs=1)
        den_d = dram.tile([H, LQ], BF, bufs=1)

        # per-slab stat columns [P tokens, tile-in-slab, (r, mu*r)]
        cols = [stat.tile([P, NTQ, 2], F32, name=f"cols{sl}", tag=f"cols{sl}",
                          bufs=1) for sl in range(NSL)]

        def stats_slab(sl):
            ps = ps1.tile([1, FD], F32, name="xs", tag="xs", bufs=1)
            ps2i = ps1.tile([1, FD], F32, name="xss", tag="xss", bufs=1)
            for d in range(ND):
                xseg = xT_sb[:, d, sl * FD:(sl + 1) * FD]
                sq = tmpA.tile([P, FD], BF, name="xsq", tag="xsq", bufs=2)
                nc.vector.tensor_mul(sq[:], xseg, xseg)
                nc.tensor.matmul(ps[:], ones_sb[:], xseg,
                                 start=(d == 0), stop=(d == ND - 1))
                nc.tensor.matmul(ps2i[:], ones_sb[:], sq[:],
                                 start=(d == 0), stop=(d == ND - 1))
            mean = stat.tile([1, FD], F32, name="xmean", tag="xmean", bufs=2)
            nc.vector.tensor_scalar_mul(mean[:], ps[:], 1.0 / D)
            vep = stat.tile([1, FD], F32, name="xvep", tag="xvep", bufs=2)
            nc.scalar.activation(vep[:], mean[:], AF.Square, bias=zb1, scale=os1)
            nc.vector.tensor_scalar(vep[:], vep[:], -1.0, EPS,
                                    op0=OP.mult, op1=OP.add)
            ex2 = stat.tile([1, FD], F32, name="xex2", tag="xex2", bufs=2)
            nc.vector.tensor_scalar_mul(ex2[:], ps2i[:], 1.0 / D)
            nc.vector.tensor_add(vep[:], vep[:], ex2[:])
            r = stat.tile([1, FD], F32, name="xr", tag="xr", bufs=2)
            nc.scalar.activation(r[:], vep[:], AF.Sqrt, bias=zb1, scale=os1)
            nc.vector.reciprocal(r[:], r[:])
            _rstd_refine(nc, stat, r, vep, (1, FD), "x", zb_sb, os_sb)
            nc.sync.dma_start(rows_d[0:1, sl * FD:(sl + 1) * FD], r[:])
            mur = stat.tile([1, FD], F32, name="xmur", tag="xmur", bufs=2)
            nc.vector.tensor_mul(mur[:], mean[:], r[:])
            nc.sync.dma_start(rows_d[1:2, sl * FD:(sl + 1) * FD], mur[:])
            # gather back as per-token columns: [p, tile, row]
            base = rows_d[:]
            for row in range(2):
                src = bass.AP(tensor=base.tensor,
                              offset=base.offset + row * L + sl * FD,
                              ap=[[1, P], [P, NTQ]])
                nc.sync.dma_start(cols[sl][:, :, row], src)

        def project(w_sb, col0, tt):
            """Two [P,512] psums of raw projection G for token tile tt."""
            pss = []
            for s2 in range(2):
                ps = ps1.tile([P, FD], F32, name=f"pj{s2}", tag=f"pj{s2}",
                              bufs=3)
                for d in range(ND):
                    nc.tensor.matmul(ps[:],
                                     xT_sb[:, d, tt * P:(tt + 1) * P],
                                     w_sb[:, d, s2 * FD:(s2 + 1) * FD],
                                     start=(d == 0), stop=(d == ND - 1))
                pss.append(ps)
            return pss

        def affine(pss, kind, tt, dst, dve_scale=False):
            """dst[:, e] = r_t * G - (mu*r)_t * s_e + c_e  (token-major)."""
            sl, ti = tt // NTQ, tt % NTQ
            rc = cols[sl][:, ti, 0:1]
            murc = cols[sl][:, ti, 1:2]
            for s2 in range(2):
                off = kind * D + s2 * FD
                corr = tmpA.tile([P, FD], BF, name="corr", tag="corr", bufs=2)
                nc.gpsimd.tensor_scalar(corr[:], sneg_sb[:, off:off + FD],
                                        murc, None, op0=OP.mult)
                nc.gpsimd.tensor_add(corr[:], corr[:],
                                     c_sb[:, off:off + FD])
                half = dst[:, s2 * FD:(s2 + 1) * FD]
                if dve_scale:
                    nc.vector.tensor_scalar(half, pss[s2][:], rc, None,
                                            op0=OP.mult)
                else:
                    nc.scalar.activation(half, pss[s2][:], AF.Identity,
                                         bias=zb, scale=rc)
                nc.vector.tensor_add(half, half, corr[:])

        def token_ln_rope(raw, w_row, scale, tt, dst_d, name, act_ln):
            st6 = stat.tile([P, 2, 6], F32, name=f"{name}bs", tag="bs", bufs=4)
            seg = raw[:].rearrange("p (s f) -> p s f", s=2)
            for s2 in range(2):
                nc.vector.bn_stats(st6[:, s2, :], seg[:, s2, :])
            mv = stat.tile([P, 2], F32, name=f"{name}mv", tag="mv", bufs=4)
            nc.vector.bn_aggr(mv[:], st6[:])
            vep = stat.tile([P, 1], F32, name=f"{name}ve", tag="ve", bufs=4)
            nc.vector.tensor_scalar(vep[:], mv[:, 1:2], 1.0, EPS,
                                    op0=OP.mult, op1=OP.add)
            r = stat.tile([P, 1], F32, name=f"{name}r", tag="lr", bufs=4)
            nc.scalar.activation(r[:], vep[:], AF.Sqrt, bias=zb, scale=os_)
            nc.vector.reciprocal(r[:], r[:])
            _rstd_refine(nc, stat, r, vep, (P, 1), "t", zb_sb, os_sb)
            if scale != 1.0:
                nc.vector.tensor_scalar_mul(r[:], r[:], scale)
            xn = tmpA.tile([P, D], BF, name=f"{name}xn", tag="xn", bufs=2)
            if act_ln:
                nmur = stat.tile([P, 1], F32, name=f"{name}nm", tag="nm",
                                 bufs=4)
                nc.vector.tensor_scalar(nmur[:], mv[:, 0:1], r, -1.0,
                                        op0=OP.mult, op1=OP.mult)
                nc.scalar.activation(xn[:], raw[:], AF.Identity,
                                     bias=nmur, scale=r)
            else:
                nc.vector.tensor_scalar(xn[:], raw[:], mv[:, 0:1], r,
                                        op0=OP.subtract, op1=OP.mult)
            nc.gpsimd.tensor_mul(xn[:], xn[:], w_row[:])
            xnv = xn[:].rearrange("p (h j) -> p h j", j=DH)
            t2 = tmpA.tile([P, H, DH], BF, name=f"{name}t2", tag="rp2",
                           bufs=1)
            nc.vector.tensor_mul(t2[:, :, 0:DH // 2],
                                 xnv[:, :, DH // 2:DH],
                                 _bc_heads(sin_sb[:, tt, 0:DH // 2], H))
            nc.vector.tensor_mul(t2[:, :, DH // 2:DH],
                                 xnv[:, :, 0:DH // 2],
                                 _bc_heads(sin_sb[:, tt, DH // 2:DH], H))
            t3 = tmpA.tile([P, H, DH], BF, name=f"{name}t3", tag="rp3",
                           bufs=1)
            nc.vector.tensor_mul(t3[:], xnv,
                                 _bc_heads(cos_sb[:, tt, :], H))
            nc.gpsimd.tensor_add(t3[:], t3[:], t2[:])
            nc.sync.dma_start(dst_d[tt * P:(tt + 1) * P, :],
                              t3[:].rearrange("p h j -> p (h j)"))

        with tc.tile_pool(name="tmpA", bufs=1) as tmpA, \
             tc.tile_pool(name="ps1", bufs=1, space="PSUM") as ps1:
            stats_slab(0)
            # ---------- Q projection + token LN + RoPE (slab 0) ----------
            for tt in range(NTQ):
                pss = project(wq_sb, 0, tt)
                raw = tmpA.tile([P, D], BF, name="qraw", tag="raw", bufs=2)
                affine(pss, 0, tt, raw, dve_scale=False)
                token_ln_rope(raw, qw_sb, DH ** -0.5, tt, qr_d, "q",
                              act_ln=False)
            nc.sync.dma_start_transpose(qrT[:], qr_d[:])
            # wv recycles wq's slot once the q tiles are done (v-proj runs
            # late, as PE filler during attention)
            wv_sb = wpool.tile([P, ND, D], BF, name="wv", tag="wslab", bufs=2)
            nc.gpsimd.dma_start(wv_sb[:], ap_wqkvT[:, :, 2 * D:3 * D])
            # ---------- K projection + token LN + RoPE, per slab ----------
            for sl in range(NSL):
                if sl + 1 < NSL:
                    stats_slab(sl + 1)
                for ti in range(NTQ):
                    tt = sl * NTQ + ti
                    pss = project(wk_sb, 1, tt)
                    raw = tmpA.tile([P, D], BF, name="kraw", tag="raw",
                                    bufs=2)
                    affine(pss, 1, tt, raw, dve_scale=False)
                    token_ln_rope(raw, kw_sb, 1.0, tt, kr_d, "k", act_ln=True)
                nc.sync.dma_start_transpose(
                    krT[:, :, sl * FD:(sl + 1) * FD],
                    kr_d[sl * FD:(sl + 1) * FD, :])
            # wo recycles wk's slot once the k tiles are done
            wo_sb = wpool.tile([P, ND, D], BF, name="wo", tag="wslab", bufs=2)
            nc.gpsimd.dma_start(wo_sb[:], ap_woutT)

        # tmpA/ps1 closed. Attention + v-proj + out-proj.
        with tc.tile_pool(name="tmpC", bufs=1) as tmpC, \
             tc.tile_pool(name="ps2", bufs=1, space="PSUM") as ps2:
            # ---------- v projection (emitted first, low priority so the
            # scheduler uses it as PE filler during attention exp waits) --
            with tc.high_priority(offset=-1000000):
                for tt in range(NT):
                    sl, ti = tt // NTQ, tt % NTQ
                    rc = cols[sl][:, ti, 0:1]
                    murc = cols[sl][:, ti, 1:2]
                    for s2 in range(2):
                        ps = ps2.tile([P, FD], F32, name="vp", tag="mm2",
                                      bufs=2)
                        for d in range(ND):
                            nc.tensor.matmul(
                                ps[:], xT_sb[:, d, tt * P:(tt + 1) * P],
                                wv_sb[:, d, s2 * FD:(s2 + 1) * FD],
                                start=(d == 0), stop=(d == ND - 1))
                        off = 2 * D + s2 * FD
                        corr = tmpC.tile([P, FD], BF, name="vcorr",
                                         tag="vcorr", bufs=2)
                        nc.gpsimd.tensor_scalar(corr[:],
                                                sneg_sb[:, off:off + FD],
                                                murc, None, op0=OP.mult)
                        nc.gpsimd.tensor_add(corr[:], corr[:],
                                             c_sb[:, off:off + FD])
                        dst = v_sb[:, tt, s2 * 8:(s2 + 1) * 8, 0:DH]
                        nc.vector.tensor_scalar(
                            dst, ps[:].rearrange("p (h e) -> p h e", e=DH),
                            rc, None, op0=OP.mult)
                        nc.vector.tensor_add(
                            dst, dst,
                            corr[:].rearrange("p (h e) -> p h e", e=DH))

            # ---------- attention (head pairs, chunked exp) -------
            for et in range(ND):
                hA, hB = 2 * et, 2 * et + 1
                ctx_a = ps2.tile([DH + 1, LQ], F32, name="ctxa", tag="ctx",
                                 bufs=2)
                ctx_b = ps2.tile([DH + 1, LQ], F32, name="ctxb", tag="ctx",
                                 bufs=2)
                kA = krT[0:DH, et, :]
                kB = krT[DH:P, et, :]
                qA = qrT[0:DH, et, :]
                qB = qrT[DH:P, et, :]
                for g in range(NT // 2):
                    st0, st1 = 2 * g, 2 * g + 1
                    spsA = ps2.tile([P, 2, LQ], F32, name="spsA",
                                    tag="sps", bufs=2)
                    spsB = ps2.tile([P, 2, LQ], F32, name="spsB",
                                    tag="sps", bufs=2)
                    nc.tensor.matmul(spsA[:, 0, :],
                                     kA[:, st0 * P:(st0 + 1) * P], qA,
                                     start=True, stop=True)
                    nc.tensor.matmul(spsB[:, 0, :],
                                     kB[:, st0 * P:(st0 + 1) * P], qB,
                                     start=True, stop=True)
                    nc.tensor.matmul(spsA[:, 1, :],
                                     kA[:, st1 * P:(st1 + 1) * P], qA,
                                     start=True, stop=True)
                    nc.tensor.matmul(spsB[:, 1, :],
                                     kB[:, st1 * P:(st1 + 1) * P], qB,
                                     start=True, stop=True)
                    expA = tmpC.tile([P, 2, LQ], BF, name="expA",
                                     tag="exp", bufs=4)
                    expB = tmpC.tile([P, 2, LQ], BF, name="expB",
                                     tag="exp", bufs=4)
                    nc.scalar.activation(expA[:], spsA[:], AF.Exp, bias=zb, scale=os_)
                    nc.scalar.activation(expB[:], spsB[:], AF.Exp, bias=zb, scale=os_)
                    for j, st in ((0, st0), (1, st1)):
                        nc.tensor.matmul(ctx_a[:], v_sb[:, st, hA, :],
                                         expA[:, j, :],
                                         start=(st == 0),
                                         stop=(st == NT - 1))
                        nc.tensor.matmul(ctx_b[:], v_sb[:, st, hB, :],
                                         expB[:, j, :],
                                         start=(st == 0),
                                         stop=(st == NT - 1))
                # denominators for both heads -> one bounce -> bc multiply
                for j, cps in ((0, ctx_a), (1, ctx_b)):
                    rrow = stat.tile([1, LQ], BF, name="rrow", tag="rrow",
                                     bufs=2)
                    with nc.allow_low_precision(reason="softmax denom"):
                        nc.vector.reciprocal(rrow[:], cps[DH:DH + 1, :])
                    nc.sync.dma_start(den_d[hA + j:hA + j + 1, :], rrow[:])
                rb = tmpC.tile([P, LQ], BF, name="rb", tag="rb", bufs=2)
                nc.sync.dma_start(rb[0:DH, :],
                                  _bc_part(den_d[hA:hA + 1, :], DH))
                nc.sync.dma_start(rb[DH:P, :],
                                  _bc_part(den_d[hB:hB + 1, :], DH))
                nc.vector.tensor_mul(ctxT[0:DH, et, :], ctx_a[0:DH, :],
                                     rb[0:DH, :])
                nc.vector.tensor_mul(ctxT[DH:P, et, :], ctx_b[0:DH, :],
                                     rb[DH:P, :])

            # ---------- output projection ----------
            for tt in range(NTQ):
                for s2 in range(2):
                    ps = ps2.tile([P, FD], F32, name="ops", tag="mm2",
                                  bufs=2)
                    for d in range(ND):
                        nc.tensor.matmul(
                            ps[:], ctxT[:, d, tt * P:(tt + 1) * P],
                            wo_sb[:, d, s2 * FD:(s2 + 1) * FD],
                            start=(d == 0), stop=(d == ND - 1))
                    o_sb = tmpC.tile([P, FD], F32, name="osb", tag="osb",
                                     bufs=2)
                    nc.scalar.activation(o_sb[:], ps[:], AF.Identity, bias=zb, scale=os_)
                    nc.sync.dma_start(
                        out.ap()[tt * P:(tt + 1) * P,
                                 s2 * FD:(s2 + 1) * FD], o_sb[:])


_NC_CACHE = None


def build_nc(do_compile=True):
    nc = bacc.Bacc("TRN2", target_bir_lowering=False, debug=False)
    _emit(nc)
    if do_compile:
        nc.compile()
    return nc


def _get_nc():
    global _NC_CACHE
    if _NC_CACHE is None:
        _NC_CACHE = build_nc(do_compile=True)
    return _NC_CACHE


def _build_tables():
    inv_freq = 1.0 / (ROPE_BASE ** (np.arange(0, DH, 2, dtype=np.float32) / DH))
    t = np.arange(L, dtype=np.float32)
    freqs = np.outer(t, inv_freq)                       # [L, 32]
    cos = np.concatenate([np.cos(freqs)] * 2, axis=1)   # [L, 64]
    sin = np.concatenate([np.sin(freqs)] * 2, axis=1)
    sign = np.where(np.arange(DH) < DH // 2, -1.0, 1.0).astype(np.float32)
    return (cos.astype(ml_dtypes.bfloat16),
            (sin * sign[None, :]).astype(ml_dtypes.bfloat16))


def make_in_maps(x, ln_w, ln_b, w_qkv, q_ln_w, k_ln_w, w_out):
    W = np.asarray(w_qkv, np.float32)                   # [3D, D]
    ln_w = np.asarray(ln_w, np.float32)
    ln_b = np.asarray(ln_b, np.float32)
    Wp = W * ln_w[None, :]                              # fold ln_w
    c = W @ ln_b                                        # [3D] fold ln_b
    s = Wp.sum(axis=1)                                  # [3D]
    wqkvT = np.ascontiguousarray(Wp.T).astype(ml_dtypes.bfloat16)
    woutT = np.ascontiguousarray(np.asarray(w_out, np.float32).T).astype(
        ml_dtypes.bfloat16)
    cos_t, sin_t = _build_tables()
    x = np.asarray(x, np.float32)
    in_maps = []
    for cid in range(NCORES):
        b, sshift = cid // 4, cid % 4
        xb = np.roll(x[b], -sshift * LQ, axis=0)
        xTc = np.ascontiguousarray(xb.T).astype(ml_dtypes.bfloat16)
        in_maps.append({
            "xT": xTc, "wqkvT": wqkvT, "woutT": woutT,
            "sneg": (-s).astype(ml_dtypes.bfloat16),
            "crow": c.astype(ml_dtypes.bfloat16),
            "q_ln_w": np.asarray(q_ln_w, np.float32).astype(ml_dtypes.bfloat16),
            "k_ln_w": np.asarray(k_ln_w, np.float32).astype(ml_dtypes.bfloat16),
            "cos_t": np.ascontiguousarray(np.roll(cos_t, -sshift * LQ, axis=0)),
            "sin_t": np.ascontiguousarray(np.roll(sin_t, -sshift * LQ, axis=0)),
        })
    return in_maps


def kernel(x, ln_w, ln_b, w_qkv, q_ln_w, k_ln_w, w_out, **run_kwargs):
    in_maps = make_in_maps(x, ln_w, ln_b, w_qkv, q_ln_w, k_ln_w, w_out)
    nc = _get_nc()
    res = run_bass_kernel_spmd(nc, in_maps, core_ids=list(range(NCORES)),
                               **run_kwargs)
    out = np.zeros((B, L, D), np.float32)
    for c in range(NCORES):
        b, s = c // 4, c % 4
        out[b, s * LQ:(s + 1) * LQ, :] = res.results[c]["out"]
    return out
